# revision 10
# baseline (speedup 1.0000x reference)
"""Multi-head causal attention (B=8, T=1024, C=1024, H=16, D=64) on 8 trn2 cores.

Sharding: data-parallel over batch B — core b computes batch element b fully
(QKV projections, causal softmax attention, output projection). No collectives.

Per-core program (all shapes hardcoded):
  xT = transpose(x)                       via PE-transpose (identity matmul)
  QT[hd, t], KT[hd, t] = W.T @ xT         (heads packed in pairs of 64 partitions)
  V[s, hd] = x @ Wv                       (natural layout, xT as lhsT)
  per head:
    S[t, s]  = QT_h.T @ KT_h  tiles       -> exp(S/8) (+row sums via accum_out)
                                          -> causal mask -> wei = E/rowsum -> DMA
    ST[s, t] = KT_h.T @ QT_h  tiles       -> exp -> mask   (E^T, unnormalized)
    ctx[t, d] = sum_s E^T[s, t-slice].T @ V[s, d]; normalized by 1/rowsum at the
                PSUM->SBUF copy (per-partition activation scale)
  ctx bounced to DRAM scratch; phase 3 reloads, PE-transposes, and computes
  out = ctx @ Wproj.T + bproj.

Above-diagonal tiles of `wei` are never written: output buffers are pre-zeroed
(donated zero buffers in the PJRT path), which the kernel relies on.
"""

import os
import sys

sys.path.insert(0, "/opt/trn_rl_repo")

from contextlib import ExitStack

import numpy as np

import concourse.bass as bass
import concourse.tile as tile
from concourse import bacc, mybir
from concourse.bass_utils import run_bass_kernel_spmd

f32 = mybir.dt.float32
f32r = mybir.dt.float32r

B, T, C, H, D = 8, 1024, 1024, 16, 64
P = 128
NT = T // P        # 8 row chunks of 128
NK = C // P        # 8 contraction chunks of 128
NW = T // 512      # 2 free-dim windows of 512
HP = H // 2        # 8 head pairs

USE_F32R = os.environ.get("KERNEL_F32R", "0") == "1"
DT = f32r if USE_F32R else f32

Exp = mybir.ActivationFunctionType.Exp
Copy = mybir.ActivationFunctionType.Copy
X_AXIS = mybir.AxisListType.X

_nc_cache = {}


def _w_dma(nc):
    return nc.sync


def build(rep: int = 1, bench_io: bool = False):
    nc = bacc.Bacc("TRN2", target_bir_lowering=False, debug=False,
                   enable_asserts=True, num_devices=8)
    x_d = nc.declare_dram_parameter("x", [T, C], f32, isOutput=False)
    wq_d = nc.declare_dram_parameter("Wq", [H, C, D], DT, isOutput=False)
    wk_d = nc.declare_dram_parameter("Wk", [H, C, D], DT, isOutput=False)
    wv_d = nc.declare_dram_parameter("Wv", [H, C, D], DT, isOutput=False)
    wp_d = nc.declare_dram_parameter("Wproj", [C, C], f32, isOutput=False)
    bp_d = nc.declare_dram_parameter("bproj", [C], f32, isOutput=False)
    id_d = nc.declare_dram_parameter("ident", [P, P], f32, isOutput=False)
    bm_d = nc.declare_dram_parameter("bandmask", [P, P], f32, isOutput=False)
    if bench_io:
        # timing-only build: big outputs land in internal DRAM (same DMA
        # cost), external output is a tiny token
        out_d = nc.dram_tensor("out", [T, C], f32)
        wei_d = nc.dram_tensor("wei", [H, T, T], f32)
        tok_d = nc.declare_dram_parameter("tok", [1, 16], f32, isOutput=True)
    else:
        out_d = nc.declare_dram_parameter("out", [T, C], f32, isOutput=True)
        wei_d = nc.declare_dram_parameter("wei", [H, T, T], f32, isOutput=True)

    ctxT_dram = nc.dram_tensor("ctxT_scratch", [C, T], DT)
    rows_dram = nc.dram_tensor("recip_rows", [H, T], f32)

    with tile.TileContext(nc) as tc, ExitStack() as top:
        glob = top.enter_context(tc.tile_pool(name="glob", bufs=1))
        id_sb = glob.tile([P, P], f32)
        nc.sync.dma_start(out=id_sb, in_=id_d[:])
        bm_sb = glob.tile([P, P], f32)
        nc.sync.dma_start(out=bm_sb, in_=bm_d[:])

        for _ in range(rep):
            _body(nc, tc, x_d, wq_d, wk_d, wv_d, wp_d, bp_d, id_sb, bm_sb,
                  out_d, wei_d, ctxT_dram, rows_dram)
        if bench_io:
            nc.sync.dma_start(out=tok_d[:], in_=id_sb[0:1, 0:16])

    nc.compile()
    return nc


def _body(nc, tc, x_d, wq_d, wk_d, wv_d, wp_d, bp_d, id_sb, bm_sb, out_d,
          wei_d, ctxT_dram, rows_dram):
    with ExitStack() as live:
        qkv = live.enter_context(tc.tile_pool(name="qkv", bufs=1))
        QT = qkv.tile([P, HP, T], DT)     # [(h%2)*64+d, head pair, t]
        KT = qkv.tile([P, HP, T], DT)
        V = qkv.tile([P, NT, H * D], DT)  # [s within chunk, s chunk, (h d)]

        # ---------------- Phase 1: x transpose + QKV projections ----------
        with ExitStack() as ph:
            px = ph.enter_context(tc.tile_pool(name="px", bufs=3))
            pxT = ph.enter_context(tc.tile_pool(name="pxT", bufs=1))
            pwv = ph.enter_context(tc.tile_pool(name="pwv", bufs=9))
            pwt = ph.enter_context(tc.tile_pool(name="pwt", bufs=16))
            pst = ph.enter_context(tc.tile_pool(name="pst", bufs=2, space="PSUM"))
            pmm = ph.enter_context(tc.tile_pool(name="pmm", bufs=3, space="PSUM"))

            xT = pxT.tile([P, NK, T], DT)  # [c within chunk, c chunk, t]
            for ti in range(NT):
                xt = px.tile([P, C], f32, tag="xchunk")
                nc.sync.dma_start(out=xt, in_=x_d[P * ti:P * (ti + 1), :])
                for ci in range(NK):
                    pt = pst.tile([P, P], f32, tag="tps")
                    nc.tensor.transpose(pt, xt[:, P * ci:P * (ci + 1)], id_sb)
                    nc.vector.tensor_copy(out=xT[:, ci, P * ti:P * (ti + 1)], in_=pt)

            # V = x @ Wv, layout [s, hd]
            for hw in range(NW):
                wvts = []
                for ci in range(NK):
                    wvt = pwv.tile([P, 8, D], DT, tag="wvt")
                    _w_dma(nc).dma_start(
                        out=wvt,
                        in_=wv_d[8 * hw:8 * hw + 8, P * ci:P * (ci + 1), :]
                        .rearrange("h c d -> c h d"))
                    wvts.append(wvt)
                for si in range(NT):
                    pt = pmm.tile([P, 512], f32, tag="mmps")
                    for ci in range(NK):
                        nc.tensor.matmul(pt, lhsT=xT[:, ci, P * si:P * (si + 1)],
                                         rhs=wvts[ci], start=(ci == 0),
                                         stop=(ci == NK - 1))
                    nc.vector.tensor_copy(out=V[:, si, 512 * hw:512 * (hw + 1)], in_=pt)

            # QT/KT = W.T @ xT, heads packed in pairs on the partition dim
            for w_d, OUT in ((wq_d, QT), (wk_d, KT)):
                for hp in range(HP):
                    wts = []
                    for ci in range(NK):
                        wt = pwt.tile([P, 2, D], DT, tag="wqk")
                        _w_dma(nc).dma_start(
                            out=wt,
                            in_=w_d[2 * hp:2 * hp + 2, P * ci:P * (ci + 1), :]
                            .rearrange("h c d -> c h d"))
                        wts.append(wt)
                    for tw in range(NW):
                        pt = pmm.tile([P, 512], f32, tag="mmps")
                        for ci in range(NK):
                            nc.tensor.matmul(pt, lhsT=wts[ci],
                                             rhs=xT[:, ci, 512 * tw:512 * (tw + 1)],
                                             start=(ci == 0), stop=(ci == NK - 1))
                        nc.vector.tensor_copy(out=OUT[:, hp, 512 * tw:512 * (tw + 1)],
                                           in_=pt)

        # ------- Phase 2: attention per head pair -------------------------
        with ExitStack() as ph:
            pcp = ph.enter_context(tc.tile_pool(name="pcp", bufs=2))
            pE = ph.enter_context(tc.tile_pool(name="pE", bufs=6))
            pET = ph.enter_context(tc.tile_pool(name="pET", bufs=4))
            pW = ph.enter_context(tc.tile_pool(name="pWout", bufs=4))
            prs = ph.enter_context(tc.tile_pool(name="prs", bufs=8))
            prc = ph.enter_context(tc.tile_pool(name="prc", bufs=2))
            prow = ph.enter_context(tc.tile_pool(name="prow", bufs=2))
            pbc = ph.enter_context(tc.tile_pool(name="pbc", bufs=2))
            psS = ph.enter_context(tc.tile_pool(name="psS", bufs=2, space="PSUM"))
            psT = ph.enter_context(tc.tile_pool(name="psT", bufs=2, space="PSUM"))
            psCT = ph.enter_context(tc.tile_pool(name="psCT", bufs=2,
                                                 space="PSUM"))

            for k in range(HP):
                # normalized ctxT rows for this head pair, [hd within pair, t]
                ctxT_pair = pcp.tile([P, T], DT, tag="cpair")
                # per-head 1/rowsum, broadcast across partitions for the
                # ctxT normalization: rows 0-63 even head, 64-127 odd head
                rbc = pbc.tile([P, T], f32, tag="rbc")
                for hl in (0, 1):
                    h = 2 * k + hl
                    po = 64 * hl
                    QTh = QT[po:po + 64, k, :]
                    KTh = KT[po:po + 64, k, :]
                    recip = prc.tile([P, NT], f32, tag="recip")

                    # --- S side: wei tiles + row sums
                    for i in range(NT):
                        jd = i // 4           # diagonal 512-window index
                        m = i % 4             # 128-band position in the window
                        band_end = 128 * m + 128
                        rs_acc = None
                        E_tiles = []
                        for j in range(jd + 1):
                            pt = psS.tile([P, 512], f32, tag="spsum")
                            nc.tensor.matmul(pt, lhsT=QTh[:, P * i:P * (i + 1)],
                                             rhs=KTh[:, 512 * j:512 * (j + 1)],
                                             start=True, stop=True)
                            E = pE.tile([P, 512], f32, tag="E")
                            rs = prs.tile([P, 1], f32, tag="rs")
                            if j < jd:
                                nc.scalar.activation(out=E, in_=pt, func=Exp,
                                                     scale=0.125, accum_out=rs)
                                rs_acc = rs
                            else:
                                # -1e30 mask on the diagonal 128-band, then
                                # exp over the valid prefix only
                                nc.vector.tensor_add(
                                    out=pt[:, 128 * m:band_end],
                                    in0=pt[:, 128 * m:band_end], in1=bm_sb)
                                nc.scalar.activation(out=E[:, :band_end],
                                                     in_=pt[:, :band_end],
                                                     func=Exp, scale=0.125,
                                                     accum_out=rs)
                                if rs_acc is not None:
                                    nc.vector.tensor_add(out=rs, in0=rs,
                                                         in1=rs_acc)
                            E_tiles.append(E)
                        nc.vector.reciprocal(out=recip[:, i:i + 1], in_=rs)
                        for j, E in enumerate(E_tiles):
                            ncols = 512 if j < jd else band_end
                            Wt = pW.tile([P, 512], f32, tag="Wt")
                            nc.vector.tensor_scalar_mul(Wt[:, :ncols],
                                                        E[:, :ncols],
                                                        recip[:, i:i + 1])
                            nc.sync.dma_start(
                                out=wei_d[h, P * i:P * (i + 1),
                                          512 * j:512 * j + ncols],
                                in_=Wt[:, :ncols])

                    # 1/rowsum as a [1, T] row (PE transpose + gather DMA),
                    # then replicate onto this head's 64 partitions
                    rp_ps = psT.tile([NT, P], f32, tag="rps")
                    nc.tensor.transpose(rp_ps, recip, id_sb)
                    rp_sb = prow.tile([NT, P], f32, tag="rpsb")
                    nc.vector.tensor_copy(out=rp_sb, in_=rp_ps)
                    nc.gpsimd.dma_start(out=rows_dram[h, :], in_=rp_sb)
                    row_ap = rows_dram[h, :]
                    nc.gpsimd.dma_start(
                        out=rbc[po:po + 64, :],
                        in_=bass.AP(tensor=row_ap.tensor, offset=row_ap.offset,
                                    ap=[[0, 64]] + [list(p) for p in row_ap.ap]))

                    # --- ST side + PV (ctxT_h = V_h^T @ E_h^T, N=512)
                    for jw in range(NW):
                        n_i = 4 * (jw + 1)
                        ct = psCT.tile([P, 512], f32, tag="ctxTps")
                        for i in range(n_i):
                            pt = psT.tile([P, 512], f32, tag="stpsum")
                            nc.tensor.matmul(pt, lhsT=KTh[:, P * i:P * (i + 1)],
                                             rhs=QTh[:, 512 * jw:512 * (jw + 1)],
                                             start=True, stop=True)
                            ET = pET.tile([P, 512], DT, tag="ET")
                            if i // 4 == jw:
                                m = i % 4
                                c0 = 128 * m
                                nc.scalar.activation(out=ET[:, c0:],
                                                     in_=pt[:, c0:],
                                                     func=Exp, scale=0.125)
                                nc.gpsimd.affine_select(
                                    out=ET[:, c0:c0 + 128],
                                    in_=ET[:, c0:c0 + 128],
                                    compare_op=mybir.AluOpType.is_ge, fill=0.0,
                                    base=0, channel_multiplier=-1,
                                    pattern=[[1, 128]])
                            else:
                                c0 = 0
                                nc.scalar.activation(out=ET, in_=pt, func=Exp,
                                                     scale=0.125)
                            # both heads of the pair ride along in lhsT (M=128
                            # costs the same as M=64); this head's 64 rows are
                            # the valid half
                            nc.tensor.matmul(ct[:, c0:],
                                             lhsT=V[:, i, P * k:P * (k + 1)],
                                             rhs=ET[:, c0:],
                                             start=(i == 0), stop=(i == n_i - 1))
                        nc.vector.tensor_mul(
                            out=ctxT_pair[po:po + 64, 512 * jw:512 * (jw + 1)],
                            in0=ct[po:po + 64, :],
                            in1=rbc[po:po + 64, 512 * jw:512 * (jw + 1)])

                nc.sync.dma_start(out=ctxT_dram[P * k:P * (k + 1), :],
                                  in_=ctxT_pair)

        # --------- Phase 3: output projection (qkv pools freed) -----------
        with ExitStack() as ph:
            px2 = ph.enter_context(tc.tile_pool(name="px2", bufs=3))
            pbig = ph.enter_context(tc.tile_pool(name="pbig", bufs=1))
            pout = ph.enter_context(tc.tile_pool(name="pout", bufs=4))
            pst2 = ph.enter_context(tc.tile_pool(name="pst2", bufs=4,
                                                 space="PSUM"))
            psP = ph.enter_context(tc.tile_pool(name="psP", bufs=2,
                                                space="PSUM"))

            WprojT = pbig.tile([P, NK, C], DT)  # [c_in, c_in chunk, c_out]
            ctxT = pbig.tile([P, NK, T], DT)    # [hd, hd chunk, t]
            for ck in range(NK):
                nc.sync.dma_start(out=ctxT[:, ck, :],
                                  in_=ctxT_dram[P * ck:P * (ck + 1), :])
            bias_bc = pbig.tile([P, C], f32)
            bp_ap = bp_d[:]
            nc.gpsimd.dma_start(
                out=bias_bc,
                in_=bass.AP(tensor=bp_ap.tensor, offset=bp_ap.offset,
                            ap=[[0, P]] + [list(p) for p in bp_ap.ap]))

            for ci in range(NK):
                chunk = px2.tile([P, C], f32, tag="chunk")
                nc.sync.dma_start(out=chunk, in_=wp_d[P * ci:P * (ci + 1), :])
                for ki in range(NK):
                    pt = pst2.tile([P, P], f32, tag="tps2")
                    nc.tensor.transpose(pt, chunk[:, P * ki:P * (ki + 1)],
                                        id_sb)
                    nc.vector.tensor_copy(out=WprojT[:, ki, P * ci:P * (ci + 1)],
                                          in_=pt)
            for ti in range(NT):
                for cw in range(NW):
                    pt = psP.tile([P, 512], f32, tag="prps")
                    for ki in range(NK):
                        nc.tensor.matmul(pt, lhsT=ctxT[:, ki, P * ti:P * (ti + 1)],
                                         rhs=WprojT[:, ki,
                                                    512 * cw:512 * (cw + 1)],
                                         start=(ki == 0), stop=(ki == NK - 1))
                    ot = pout.tile([P, 512], f32, tag="ot")
                    nc.vector.tensor_add(out=ot, in0=pt,
                                         in1=bias_bc[:, 512 * cw:512 * (cw + 1)])
                    nc.sync.dma_start(
                        out=out_d[P * ti:P * (ti + 1), 512 * cw:512 * (cw + 1)],
                        in_=ot)


def kernel(x, Wq, Wk, Wv, Wproj, bproj):
    x = np.ascontiguousarray(np.asarray(x, dtype=np.float32))
    Wq = np.ascontiguousarray(np.asarray(Wq, dtype=np.float32))
    Wk = np.ascontiguousarray(np.asarray(Wk, dtype=np.float32))
    Wv = np.ascontiguousarray(np.asarray(Wv, dtype=np.float32))
    Wproj = np.ascontiguousarray(np.asarray(Wproj, dtype=np.float32))
    bproj = np.ascontiguousarray(np.asarray(bproj, dtype=np.float32))

    if "nc" not in _nc_cache:
        _nc_cache["nc"] = build()
    nc = _nc_cache["nc"]

    ident = np.eye(P, dtype=np.float32)
    bandmask = np.where(np.arange(P)[None, :] <= np.arange(P)[:, None],
                        0.0, -1e30).astype(np.float32)
    in_maps = [
        {"x": x[b], "Wq": Wq, "Wk": Wk, "Wv": Wv, "Wproj": Wproj,
         "bproj": bproj, "ident": ident, "bandmask": bandmask}
        for b in range(B)
    ]
    res = run_bass_kernel_spmd(nc, in_maps, list(range(B)))
    out = np.stack([res.results[b]["out"] for b in range(B)])
    wei = np.stack([res.results[b]["wei"] for b in range(B)])
    return (out, wei)


# revision 15
# speedup vs baseline: 1.1067x; 1.1067x over previous
"""Multi-head causal attention (B=8, T=1024, C=1024, H=16, D=64) on 8 trn2 cores.

Sharding: data-parallel over batch B — core b computes batch element b fully
(QKV projections, causal softmax attention, output projection). No collectives.

Per-core program (all shapes hardcoded):
  xT = transpose(x)                       via PE-transpose (identity matmul)
  QT[hd, t], KT[hd, t] = W.T @ xT         (heads packed in pairs of 64 partitions)
  V[s, hd] = x @ Wv                       (natural layout, xT as lhsT)
  per head:
    S[t, s]  = QT_h.T @ KT_h  tiles       -> exp(S/8) (+row sums via accum_out)
                                          -> causal mask -> wei = E/rowsum -> DMA
    ST[s, t] = KT_h.T @ QT_h  tiles       -> exp -> mask   (E^T, unnormalized)
    ctx[t, d] = sum_s E^T[s, t-slice].T @ V[s, d]; normalized by 1/rowsum at the
                PSUM->SBUF copy (per-partition activation scale)
  ctx bounced to DRAM scratch; phase 3 reloads, PE-transposes, and computes
  out = ctx @ Wproj.T + bproj.

Above-diagonal tiles of `wei` are never written: output buffers are pre-zeroed
(donated zero buffers in the PJRT path), which the kernel relies on.
"""

import os
import sys

sys.path.insert(0, "/opt/trn_rl_repo")

from contextlib import ExitStack

import numpy as np

import concourse.bass as bass
import concourse.tile as tile
from concourse import bacc, mybir
from concourse.bass_utils import run_bass_kernel_spmd

f32 = mybir.dt.float32
f32r = mybir.dt.float32r

B, T, C, H, D = 8, 1024, 1024, 16, 64
P = 128
NT = T // P        # 8 row chunks of 128
NK = C // P        # 8 contraction chunks of 128
NW = T // 512      # 2 free-dim windows of 512
HP = H // 2        # 8 head pairs

USE_F32R = os.environ.get("KERNEL_F32R", "0") == "1"
DT = f32r if USE_F32R else f32

Exp = mybir.ActivationFunctionType.Exp
Copy = mybir.ActivationFunctionType.Copy
X_AXIS = mybir.AxisListType.X

_nc_cache = {}


def _w_dma(nc):
    return nc.sync


def build(rep: int = 1, bench_io: bool = False):
    nc = bacc.Bacc("TRN2", target_bir_lowering=False, debug=False,
                   enable_asserts=True, num_devices=8)
    x_d = nc.declare_dram_parameter("x", [T, C], f32, isOutput=False)
    wq_d = nc.declare_dram_parameter("Wq", [H, C, D], DT, isOutput=False)
    wk_d = nc.declare_dram_parameter("Wk", [H, C, D], DT, isOutput=False)
    wv_d = nc.declare_dram_parameter("Wv", [H, C, D], DT, isOutput=False)
    wp_d = nc.declare_dram_parameter("Wproj", [C, C], f32, isOutput=False)
    bp_d = nc.declare_dram_parameter("bproj", [C], f32, isOutput=False)
    id_d = nc.declare_dram_parameter("ident", [P, P], f32, isOutput=False)
    bm_d = nc.declare_dram_parameter("bandmask", [P, P], f32, isOutput=False)
    if bench_io:
        # timing-only build: big outputs land in internal DRAM (same DMA
        # cost), external output is a tiny token
        out_d = nc.dram_tensor("out", [T, C], f32)
        wei_d = nc.dram_tensor("wei", [H, T, T], f32)
        tok_d = nc.declare_dram_parameter("tok", [1, 16], f32, isOutput=True)
    else:
        out_d = nc.declare_dram_parameter("out", [T, C], f32, isOutput=True)
        wei_d = nc.declare_dram_parameter("wei", [H, T, T], f32, isOutput=True)

    ctxT_dram = nc.dram_tensor("ctxT_scratch", [C, T], DT)
    rows_dram = nc.dram_tensor("recip_rows", [H, T], f32)

    with tile.TileContext(nc) as tc, ExitStack() as top:
        glob = top.enter_context(tc.tile_pool(name="glob", bufs=1))
        id_sb = glob.tile([P, P], f32)
        nc.sync.dma_start(out=id_sb, in_=id_d[:])
        bm_sb = glob.tile([P, P], f32)
        nc.sync.dma_start(out=bm_sb, in_=bm_d[:])

        for _ in range(rep):
            _body(nc, tc, x_d, wq_d, wk_d, wv_d, wp_d, bp_d, id_sb, bm_sb,
                  out_d, wei_d, ctxT_dram, rows_dram)
        if bench_io:
            nc.sync.dma_start(out=tok_d[:], in_=id_sb[0:1, 0:16])

    nc.compile()
    return nc


def _body(nc, tc, x_d, wq_d, wk_d, wv_d, wp_d, bp_d, id_sb, bm_sb, out_d,
          wei_d, ctxT_dram, rows_dram):
    with ExitStack() as live:
        qkv = live.enter_context(tc.tile_pool(name="qkv", bufs=1))
        V = qkv.tile([P, NT, H * D], DT)  # [s within chunk, s chunk, (h d)]
        xT = qkv.tile([P, NK, T], DT)     # [c within chunk, c chunk, t]

        # ------- Phase 1a: x transpose + V projection ----------------------
        with ExitStack() as ph:
            px = ph.enter_context(tc.tile_pool(name="px", bufs=2))
            pst = ph.enter_context(tc.tile_pool(name="pst", bufs=2, space="PSUM"))

            for ti in range(NT):
                xt = px.tile([P, C], f32, tag="xchunk")
                nc.sync.dma_start(out=xt, in_=x_d[P * ti:P * (ti + 1), :])
                for ci in range(NK):
                    pt = pst.tile([P, P], f32, tag="tps")
                    nc.tensor.transpose(pt, xt[:, P * ci:P * (ci + 1)], id_sb)
                    nc.vector.tensor_copy(out=xT[:, ci, P * ti:P * (ti + 1)],
                                          in_=pt)

        # ------- Phase 2: per pair: Q/K projection then attention ----------
        with ExitStack() as ph:
            pqk = ph.enter_context(tc.tile_pool(name="pqk", bufs=2))
            pwv = ph.enter_context(tc.tile_pool(name="pwv", bufs=9))
            pwt = ph.enter_context(tc.tile_pool(name="pwt", bufs=16))
            pcp = ph.enter_context(tc.tile_pool(name="pcp", bufs=2))
            pE = ph.enter_context(tc.tile_pool(name="pE", bufs=8))
            pET = ph.enter_context(tc.tile_pool(name="pET", bufs=6))
            pW = ph.enter_context(tc.tile_pool(name="pWout", bufs=3))
            prs = ph.enter_context(tc.tile_pool(name="prs", bufs=10))
            prc = ph.enter_context(tc.tile_pool(name="prc", bufs=4))
            prow = ph.enter_context(tc.tile_pool(name="prow", bufs=2))
            pbc = ph.enter_context(tc.tile_pool(name="pbc", bufs=2))
            psQK = ph.enter_context(tc.tile_pool(name="psQK", bufs=2,
                                                 space="PSUM"))
            psM = ph.enter_context(tc.tile_pool(name="psM", bufs=4, space="PSUM"))
            psCT = ph.enter_context(tc.tile_pool(name="psCT", bufs=2,
                                                 space="PSUM"))

            def project_qk(k):
                QTp = pqk.tile([P, T], DT, tag="qt", name=f"qt{k}")
                KTp = pqk.tile([P, T], DT, tag="kt", name=f"kt{k}")
                for w_d, OUT in ((wq_d, QTp), (wk_d, KTp)):
                    wts = []
                    for ci in range(NK):
                        wt = pwt.tile([P, 2, D], DT, tag="wqk")
                        nc.sync.dma_start(
                            out=wt,
                            in_=w_d[2 * k:2 * k + 2, P * ci:P * (ci + 1), :]
                            .rearrange("h c d -> c h d"))
                        wts.append(wt)
                    for tw in range(NW):
                        pt = psQK.tile([P, 512], f32, tag="qkps")
                        for ci in range(NK):
                            nc.tensor.matmul(pt, lhsT=wts[ci],
                                             rhs=xT[:, ci, 512 * tw:512 * (tw + 1)],
                                             start=(ci == 0), stop=(ci == NK - 1))
                        nc.vector.tensor_copy(
                            out=OUT[:, 512 * tw:512 * (tw + 1)], in_=pt)
                return QTp, KTp

            def compute_v(hw):
                wvts = []
                for ci in range(NK):
                    wvt = pwv.tile([P, 8, D], DT, tag="wvt")
                    nc.sync.dma_start(
                        out=wvt,
                        in_=wv_d[8 * hw:8 * hw + 8, P * ci:P * (ci + 1), :]
                        .rearrange("h c d -> c h d"))
                    wvts.append(wvt)
                for si in range(NT):
                    pt = psQK.tile([P, 512], f32, tag="qkps")
                    for ci in range(NK):
                        nc.tensor.matmul(pt, lhsT=xT[:, ci, P * si:P * (si + 1)],
                                         rhs=wvts[ci], start=(ci == 0),
                                         stop=(ci == NK - 1))
                    nc.vector.tensor_copy(out=V[:, si, 512 * hw:512 * (hw + 1)],
                                          in_=pt)

            compute_v(0)
            pend = project_qk(0)
            compute_v(1)
            for k in range(HP):
                QTp, KTp = pend
                if k + 1 < HP:
                    pend = project_qk(k + 1)

                # normalized ctxT rows for this head pair, [hd within pair, t]
                ctxT_pair = pcp.tile([P, T], DT, tag="cpair")
                # per-head 1/rowsum broadcast rows: 0-63 even, 64-127 odd
                rbc = pbc.tile([P, T], f32, tag="rbc")
                QThs = [QTp[64 * hl:64 * hl + 64, :] for hl in (0, 1)]
                KThs = [KTp[64 * hl:64 * hl + 64, :] for hl in (0, 1)]
                recips = [prc.tile([P, NT], f32, tag="recip",
                                   name=f"recip{k}_{hl}") for hl in (0, 1)]

                # --- S side, both heads interleaved (concurrent row groups)
                for i in range(NT):
                    jd = i // 4           # diagonal 512-window index
                    m = i % 4             # 128-band position in the window
                    band_end = 128 * m + 128
                    rs_fin = [None, None]
                    E_all = [[], []]
                    for j in range(jd + 1):
                        pts = []
                        for hl in (0, 1):
                            pt = psM.tile([P, 512], f32, tag="mps")
                            nc.tensor.matmul(
                                pt, lhsT=QThs[hl][:, P * i:P * (i + 1)],
                                rhs=KThs[hl][:, 512 * j:512 * (j + 1)],
                                start=True, stop=True)
                            pts.append(pt)
                        for hl in (0, 1):
                            pt = pts[hl]
                            E = pE.tile([P, 512], f32, tag="E")
                            rs = prs.tile([P, 1], f32, tag="rs")
                            if j < jd:
                                nc.scalar.activation(out=E, in_=pt, func=Exp,
                                                     scale=0.125, accum_out=rs)
                            else:
                                nc.vector.tensor_add(
                                    out=pt[:, 128 * m:band_end],
                                    in0=pt[:, 128 * m:band_end], in1=bm_sb)
                                nc.scalar.activation(out=E[:, :band_end],
                                                     in_=pt[:, :band_end],
                                                     func=Exp, scale=0.125,
                                                     accum_out=rs)
                                if rs_fin[hl] is not None:
                                    nc.vector.tensor_add(out=rs, in0=rs,
                                                         in1=rs_fin[hl])
                            rs_fin[hl] = rs
                            E_all[hl].append(E)
                    for hl in (0, 1):
                        nc.vector.reciprocal(out=recips[hl][:, i:i + 1],
                                             in_=rs_fin[hl])
                        Wt = pW.tile([P, T], f32, tag="Wt")
                        for j, E in enumerate(E_all[hl]):
                            ncols = 512 if j < jd else band_end
                            nc.vector.tensor_scalar_mul(
                                Wt[:, 512 * j:512 * j + ncols], E[:, :ncols],
                                recips[hl][:, i:i + 1])
                        width = 512 * jd + band_end
                        nc.sync.dma_start(
                            out=wei_d[2 * k + hl, P * i:P * (i + 1), :width],
                            in_=Wt[:, :width])

                # 1/rowsum -> [1, T] rows in DRAM -> partition-broadcast
                for hl in (0, 1):
                    rp_ps = psM.tile([P, 512], f32, tag="mps",
                                     name=f"rps{k}_{hl}")
                    nc.tensor.transpose(rp_ps[:NT, :P], recips[hl], id_sb)
                    rp_sb = prow.tile([NT, P], f32, tag="rpsb")
                    nc.vector.tensor_copy(out=rp_sb, in_=rp_ps[:NT, :P])
                    nc.gpsimd.dma_start(out=rows_dram[2 * k + hl, :], in_=rp_sb)
                    row_ap = rows_dram[2 * k + hl, :]
                    nc.gpsimd.dma_start(
                        out=rbc[64 * hl:64 * hl + 64, :],
                        in_=bass.AP(tensor=row_ap.tensor, offset=row_ap.offset,
                                    ap=[[0, 64]] + [list(p) for p in row_ap.ap]))

                # --- ST side + PV, heads interleaved
                for jw in range(NW):
                    n_i = 4 * (jw + 1)
                    cts = [psCT.tile([P, 512], f32, tag="ctps",
                                     name=f"ct{k}_{jw}_{hl}") for hl in (0, 1)]
                    for i in range(n_i):
                        pts = []
                        for hl in (0, 1):
                            pt = psM.tile([P, 512], f32, tag="mps")
                            nc.tensor.matmul(
                                pt, lhsT=KThs[hl][:, P * i:P * (i + 1)],
                                rhs=QThs[hl][:, 512 * jw:512 * (jw + 1)],
                                start=True, stop=True)
                            pts.append(pt)
                        diag = i // 4 == jw
                        c0 = 128 * (i % 4) if diag else 0
                        ETs = []
                        for hl in (0, 1):
                            ET = pET.tile([P, 512], DT, tag="ET")
                            nc.scalar.activation(out=ET[:, c0:],
                                                 in_=pts[hl][:, c0:],
                                                 func=Exp, scale=0.125)
                            if diag:
                                nc.gpsimd.affine_select(
                                    out=ET[:, c0:c0 + 128],
                                    in_=ET[:, c0:c0 + 128],
                                    compare_op=mybir.AluOpType.is_ge, fill=0.0,
                                    base=0, channel_multiplier=-1,
                                    pattern=[[1, 128]])
                            ETs.append(ET)
                        # M=128 lhsT covers both heads; each head's valid half
                        # lands on its own partitions
                        for hl in (0, 1):
                            nc.tensor.matmul(cts[hl][:, c0:],
                                             lhsT=V[:, i, P * k:P * (k + 1)],
                                             rhs=ETs[hl][:, c0:],
                                             start=(i == 0), stop=(i == n_i - 1))
                    for hl in (0, 1):
                        po = 64 * hl
                        nc.vector.tensor_mul(
                            out=ctxT_pair[po:po + 64, 512 * jw:512 * (jw + 1)],
                            in0=cts[hl][po:po + 64, :],
                            in1=rbc[po:po + 64, 512 * jw:512 * (jw + 1)])
                nc.sync.dma_start(out=ctxT_dram[P * k:P * (k + 1), :],
                                  in_=ctxT_pair)

        # --------- Phase 3: output projection (qkv pools freed) -----------
        with ExitStack() as ph:
            px2 = ph.enter_context(tc.tile_pool(name="px2", bufs=3))
            pbig = ph.enter_context(tc.tile_pool(name="pbig", bufs=1))
            pout = ph.enter_context(tc.tile_pool(name="pout", bufs=4))
            pst2 = ph.enter_context(tc.tile_pool(name="pst2", bufs=4,
                                                 space="PSUM"))
            psP = ph.enter_context(tc.tile_pool(name="psP", bufs=2,
                                                space="PSUM"))

            WprojT = pbig.tile([P, NK, C], DT)  # [c_in, c_in chunk, c_out]
            ctxT = pbig.tile([P, NK, T], DT)    # [hd, hd chunk, t]
            for ck in range(NK):
                nc.sync.dma_start(out=ctxT[:, ck, :],
                                  in_=ctxT_dram[P * ck:P * (ck + 1), :])
            bias_bc = pbig.tile([P, C], f32)
            bp_ap = bp_d[:]
            nc.gpsimd.dma_start(
                out=bias_bc,
                in_=bass.AP(tensor=bp_ap.tensor, offset=bp_ap.offset,
                            ap=[[0, P]] + [list(p) for p in bp_ap.ap]))

            for ci in range(NK):
                chunk = px2.tile([P, C], f32, tag="chunk")
                nc.sync.dma_start(out=chunk, in_=wp_d[P * ci:P * (ci + 1), :])
                for ki in range(NK):
                    pt = pst2.tile([P, P], f32, tag="tps2")
                    nc.tensor.transpose(pt, chunk[:, P * ki:P * (ki + 1)],
                                        id_sb)
                    nc.vector.tensor_copy(out=WprojT[:, ki, P * ci:P * (ci + 1)],
                                          in_=pt)
            for ti in range(NT):
                for cw in range(NW):
                    pt = psP.tile([P, 512], f32, tag="prps")
                    for ki in range(NK):
                        nc.tensor.matmul(pt, lhsT=ctxT[:, ki, P * ti:P * (ti + 1)],
                                         rhs=WprojT[:, ki,
                                                    512 * cw:512 * (cw + 1)],
                                         start=(ki == 0), stop=(ki == NK - 1))
                    ot = pout.tile([P, 512], f32, tag="ot")
                    nc.vector.tensor_add(out=ot, in0=pt,
                                         in1=bias_bc[:, 512 * cw:512 * (cw + 1)])
                    nc.sync.dma_start(
                        out=out_d[P * ti:P * (ti + 1), 512 * cw:512 * (cw + 1)],
                        in_=ot)


def kernel(x, Wq, Wk, Wv, Wproj, bproj):
    x = np.ascontiguousarray(np.asarray(x, dtype=np.float32))
    Wq = np.ascontiguousarray(np.asarray(Wq, dtype=np.float32))
    Wk = np.ascontiguousarray(np.asarray(Wk, dtype=np.float32))
    Wv = np.ascontiguousarray(np.asarray(Wv, dtype=np.float32))
    Wproj = np.ascontiguousarray(np.asarray(Wproj, dtype=np.float32))
    bproj = np.ascontiguousarray(np.asarray(bproj, dtype=np.float32))

    if "nc" not in _nc_cache:
        _nc_cache["nc"] = build()
    nc = _nc_cache["nc"]

    ident = np.eye(P, dtype=np.float32)
    bandmask = np.where(np.arange(P)[None, :] <= np.arange(P)[:, None],
                        0.0, -1e30).astype(np.float32)
    in_maps = [
        {"x": x[b], "Wq": Wq, "Wk": Wk, "Wv": Wv, "Wproj": Wproj,
         "bproj": bproj, "ident": ident, "bandmask": bandmask}
        for b in range(B)
    ]
    res = run_bass_kernel_spmd(nc, in_maps, list(range(B)))
    out = np.stack([res.results[b]["out"] for b in range(B)])
    wei = np.stack([res.results[b]["wei"] for b in range(B)])
    return (out, wei)


# revision 16
# speedup vs baseline: 3917.1701x; 3539.5153x over previous
"""Multi-head causal attention (B=8, T=1024, C=1024, H=16, D=64) on 8 trn2 cores.

Sharding: data-parallel over batch B — core b computes batch element b fully
(QKV projections, causal softmax attention, output projection). No collectives.

Per-core program (all shapes hardcoded):
  xT = transpose(x)                       via PE-transpose (identity matmul)
  QT[hd, t], KT[hd, t] = W.T @ xT         (heads packed in pairs of 64 partitions)
  V[s, hd] = x @ Wv                       (natural layout, xT as lhsT)
  per head:
    S[t, s]  = QT_h.T @ KT_h  tiles       -> exp(S/8) (+row sums via accum_out)
                                          -> causal mask -> wei = E/rowsum -> DMA
    ST[s, t] = KT_h.T @ QT_h  tiles       -> exp -> mask   (E^T, unnormalized)
    ctx[t, d] = sum_s E^T[s, t-slice].T @ V[s, d]; normalized by 1/rowsum at the
                PSUM->SBUF copy (per-partition activation scale)
  ctx bounced to DRAM scratch; phase 3 reloads, PE-transposes, and computes
  out = ctx @ Wproj.T + bproj.

Above-diagonal tiles of `wei` are never written: output buffers are pre-zeroed
(donated zero buffers in the PJRT path), which the kernel relies on.
"""

import os
import sys

sys.path.insert(0, "/opt/trn_rl_repo")

from contextlib import ExitStack

import numpy as np

import concourse.bass as bass
import concourse.tile as tile
from concourse import bacc, mybir
from concourse.bass_utils import run_bass_kernel_spmd

f32 = mybir.dt.float32
f32r = mybir.dt.float32r

B, T, C, H, D = 8, 1024, 1024, 16, 64
P = 128
NT = T // P        # 8 row chunks of 128
NK = C // P        # 8 contraction chunks of 128
NW = T // 512      # 2 free-dim windows of 512
HP = H // 2        # 8 head pairs

USE_F32R = os.environ.get("KERNEL_F32R", "1") == "1"
DT = f32r if USE_F32R else f32

Exp = mybir.ActivationFunctionType.Exp
Copy = mybir.ActivationFunctionType.Copy
X_AXIS = mybir.AxisListType.X

_nc_cache = {}


def _w_dma(nc):
    return nc.sync


def build(rep: int = 1, bench_io: bool = False):
    nc = bacc.Bacc("TRN2", target_bir_lowering=False, debug=False,
                   enable_asserts=True, num_devices=8)
    x_d = nc.declare_dram_parameter("x", [T, C], f32, isOutput=False)
    wq_d = nc.declare_dram_parameter("Wq", [H, C, D], DT, isOutput=False)
    wk_d = nc.declare_dram_parameter("Wk", [H, C, D], DT, isOutput=False)
    wv_d = nc.declare_dram_parameter("Wv", [H, C, D], DT, isOutput=False)
    wp_d = nc.declare_dram_parameter("Wproj", [C, C], f32, isOutput=False)
    bp_d = nc.declare_dram_parameter("bproj", [C], f32, isOutput=False)
    id_d = nc.declare_dram_parameter("ident", [P, P], f32, isOutput=False)
    bm_d = nc.declare_dram_parameter("bandmask", [P, P], f32, isOutput=False)
    if bench_io:
        # timing-only build: big outputs land in internal DRAM (same DMA
        # cost), external output is a tiny token
        out_d = nc.dram_tensor("out", [T, C], f32)
        wei_d = nc.dram_tensor("wei", [H, T, T], f32)
        tok_d = nc.declare_dram_parameter("tok", [1, 16], f32, isOutput=True)
    else:
        out_d = nc.declare_dram_parameter("out", [T, C], f32, isOutput=True)
        wei_d = nc.declare_dram_parameter("wei", [H, T, T], f32, isOutput=True)

    ctxT_dram = nc.dram_tensor("ctxT_scratch", [C, T], DT)
    rows_dram = nc.dram_tensor("recip_rows", [H, T], f32)

    with tile.TileContext(nc) as tc, ExitStack() as top:
        glob = top.enter_context(tc.tile_pool(name="glob", bufs=1))
        id_sb = glob.tile([P, P], f32)
        nc.sync.dma_start(out=id_sb, in_=id_d[:])
        bm_sb = glob.tile([P, P], f32)
        nc.sync.dma_start(out=bm_sb, in_=bm_d[:])

        for _ in range(rep):
            _body(nc, tc, x_d, wq_d, wk_d, wv_d, wp_d, bp_d, id_sb, bm_sb,
                  out_d, wei_d, ctxT_dram, rows_dram)
        if bench_io:
            nc.sync.dma_start(out=tok_d[:], in_=id_sb[0:1, 0:16])

    nc.compile()
    return nc


def _body(nc, tc, x_d, wq_d, wk_d, wv_d, wp_d, bp_d, id_sb, bm_sb, out_d,
          wei_d, ctxT_dram, rows_dram):
    with ExitStack() as live:
        qkv = live.enter_context(tc.tile_pool(name="qkv", bufs=1))
        V = qkv.tile([P, NT, H * D], DT)  # [s within chunk, s chunk, (h d)]
        xT = qkv.tile([P, NK, T], DT)     # [c within chunk, c chunk, t]

        # ------- Phase 1a: x transpose + V projection ----------------------
        with ExitStack() as ph:
            px = ph.enter_context(tc.tile_pool(name="px", bufs=2))
            pst = ph.enter_context(tc.tile_pool(name="pst", bufs=2, space="PSUM"))

            for ti in range(NT):
                xt = px.tile([P, C], f32, tag="xchunk")
                nc.sync.dma_start(out=xt, in_=x_d[P * ti:P * (ti + 1), :])
                for ci in range(NK):
                    pt = pst.tile([P, P], f32, tag="tps")
                    nc.tensor.transpose(pt, xt[:, P * ci:P * (ci + 1)], id_sb)
                    nc.vector.tensor_copy(out=xT[:, ci, P * ti:P * (ti + 1)],
                                          in_=pt)

        # ------- Phase 2: per pair: Q/K projection then attention ----------
        with ExitStack() as ph:
            pqk = ph.enter_context(tc.tile_pool(name="pqk", bufs=2))
            pwv = ph.enter_context(tc.tile_pool(name="pwv", bufs=9))
            pwt = ph.enter_context(tc.tile_pool(name="pwt", bufs=16))
            pcp = ph.enter_context(tc.tile_pool(name="pcp", bufs=2))
            pE = ph.enter_context(tc.tile_pool(name="pE", bufs=8))
            pET = ph.enter_context(tc.tile_pool(name="pET", bufs=6))
            pW = ph.enter_context(tc.tile_pool(name="pWout", bufs=3))
            prs = ph.enter_context(tc.tile_pool(name="prs", bufs=10))
            prc = ph.enter_context(tc.tile_pool(name="prc", bufs=4))
            prow = ph.enter_context(tc.tile_pool(name="prow", bufs=2))
            pbc = ph.enter_context(tc.tile_pool(name="pbc", bufs=2))
            psQK = ph.enter_context(tc.tile_pool(name="psQK", bufs=2,
                                                 space="PSUM"))
            psM = ph.enter_context(tc.tile_pool(name="psM", bufs=4, space="PSUM"))
            psCT = ph.enter_context(tc.tile_pool(name="psCT", bufs=2,
                                                 space="PSUM"))

            def project_qk(k):
                QTp = pqk.tile([P, T], DT, tag="qt", name=f"qt{k}")
                KTp = pqk.tile([P, T], DT, tag="kt", name=f"kt{k}")
                for w_d, OUT in ((wq_d, QTp), (wk_d, KTp)):
                    wts = []
                    for ci in range(NK):
                        wt = pwt.tile([P, 2, D], DT, tag="wqk")
                        nc.sync.dma_start(
                            out=wt,
                            in_=w_d[2 * k:2 * k + 2, P * ci:P * (ci + 1), :]
                            .rearrange("h c d -> c h d"))
                        wts.append(wt)
                    for tw in range(NW):
                        pt = psQK.tile([P, 512], f32, tag="qkps")
                        for ci in range(NK):
                            nc.tensor.matmul(pt, lhsT=wts[ci],
                                             rhs=xT[:, ci, 512 * tw:512 * (tw + 1)],
                                             start=(ci == 0), stop=(ci == NK - 1))
                        nc.vector.tensor_copy(
                            out=OUT[:, 512 * tw:512 * (tw + 1)], in_=pt)
                return QTp, KTp

            def compute_v(hw):
                wvts = []
                for ci in range(NK):
                    wvt = pwv.tile([P, 8, D], DT, tag="wvt")
                    nc.sync.dma_start(
                        out=wvt,
                        in_=wv_d[8 * hw:8 * hw + 8, P * ci:P * (ci + 1), :]
                        .rearrange("h c d -> c h d"))
                    wvts.append(wvt)
                for si in range(NT):
                    pt = psQK.tile([P, 512], f32, tag="qkps")
                    for ci in range(NK):
                        nc.tensor.matmul(pt, lhsT=xT[:, ci, P * si:P * (si + 1)],
                                         rhs=wvts[ci], start=(ci == 0),
                                         stop=(ci == NK - 1))
                    nc.vector.tensor_copy(out=V[:, si, 512 * hw:512 * (hw + 1)],
                                          in_=pt)

            compute_v(0)
            pend = project_qk(0)
            compute_v(1)
            for k in range(HP):
                QTp, KTp = pend
                if k + 1 < HP:
                    pend = project_qk(k + 1)

                # normalized ctxT rows for this head pair, [hd within pair, t]
                ctxT_pair = pcp.tile([P, T], DT, tag="cpair")
                # per-head 1/rowsum broadcast rows: 0-63 even, 64-127 odd
                rbc = pbc.tile([P, T], f32, tag="rbc")
                QThs = [QTp[64 * hl:64 * hl + 64, :] for hl in (0, 1)]
                KThs = [KTp[64 * hl:64 * hl + 64, :] for hl in (0, 1)]
                recips = [prc.tile([P, NT], f32, tag="recip",
                                   name=f"recip{k}_{hl}") for hl in (0, 1)]

                # --- S side, both heads interleaved (concurrent row groups)
                for i in range(NT):
                    jd = i // 4           # diagonal 512-window index
                    m = i % 4             # 128-band position in the window
                    band_end = 128 * m + 128
                    rs_fin = [None, None]
                    E_all = [[], []]
                    for j in range(jd + 1):
                        pts = []
                        for hl in (0, 1):
                            pt = psM.tile([P, 512], f32, tag="mps")
                            nc.tensor.matmul(
                                pt, lhsT=QThs[hl][:, P * i:P * (i + 1)],
                                rhs=KThs[hl][:, 512 * j:512 * (j + 1)],
                                start=True, stop=True)
                            pts.append(pt)
                        for hl in (0, 1):
                            pt = pts[hl]
                            E = pE.tile([P, 512], f32, tag="E")
                            rs = prs.tile([P, 1], f32, tag="rs")
                            if j < jd:
                                nc.scalar.activation(out=E, in_=pt, func=Exp,
                                                     scale=0.125, accum_out=rs)
                            else:
                                nc.vector.tensor_add(
                                    out=pt[:, 128 * m:band_end],
                                    in0=pt[:, 128 * m:band_end], in1=bm_sb)
                                nc.scalar.activation(out=E[:, :band_end],
                                                     in_=pt[:, :band_end],
                                                     func=Exp, scale=0.125,
                                                     accum_out=rs)
                                if rs_fin[hl] is not None:
                                    nc.vector.tensor_add(out=rs, in0=rs,
                                                         in1=rs_fin[hl])
                            rs_fin[hl] = rs
                            E_all[hl].append(E)
                    for hl in (0, 1):
                        nc.vector.reciprocal(out=recips[hl][:, i:i + 1],
                                             in_=rs_fin[hl])
                        Wt = pW.tile([P, T], f32, tag="Wt")
                        for j, E in enumerate(E_all[hl]):
                            ncols = 512 if j < jd else band_end
                            nc.vector.tensor_scalar_mul(
                                Wt[:, 512 * j:512 * j + ncols], E[:, :ncols],
                                recips[hl][:, i:i + 1])
                        width = 512 * jd + band_end
                        nc.sync.dma_start(
                            out=wei_d[2 * k + hl, P * i:P * (i + 1), :width],
                            in_=Wt[:, :width])

                # 1/rowsum -> [1, T] rows in DRAM -> partition-broadcast
                for hl in (0, 1):
                    rp_ps = psM.tile([P, 512], f32, tag="mps",
                                     name=f"rps{k}_{hl}")
                    nc.tensor.transpose(rp_ps[:NT, :P], recips[hl], id_sb)
                    rp_sb = prow.tile([NT, P], f32, tag="rpsb")
                    nc.vector.tensor_copy(out=rp_sb, in_=rp_ps[:NT, :P])
                    nc.gpsimd.dma_start(out=rows_dram[2 * k + hl, :], in_=rp_sb)
                    row_ap = rows_dram[2 * k + hl, :]
                    nc.gpsimd.dma_start(
                        out=rbc[64 * hl:64 * hl + 64, :],
                        in_=bass.AP(tensor=row_ap.tensor, offset=row_ap.offset,
                                    ap=[[0, 64]] + [list(p) for p in row_ap.ap]))

                # --- ST side + PV, heads interleaved
                for jw in range(NW):
                    n_i = 4 * (jw + 1)
                    cts = [psCT.tile([P, 512], f32, tag="ctps",
                                     name=f"ct{k}_{jw}_{hl}") for hl in (0, 1)]
                    for i in range(n_i):
                        pts = []
                        for hl in (0, 1):
                            pt = psM.tile([P, 512], f32, tag="mps")
                            nc.tensor.matmul(
                                pt, lhsT=KThs[hl][:, P * i:P * (i + 1)],
                                rhs=QThs[hl][:, 512 * jw:512 * (jw + 1)],
                                start=True, stop=True)
                            pts.append(pt)
                        diag = i // 4 == jw
                        c0 = 128 * (i % 4) if diag else 0
                        ETs = []
                        for hl in (0, 1):
                            ET = pET.tile([P, 512], DT, tag="ET")
                            nc.scalar.activation(out=ET[:, c0:],
                                                 in_=pts[hl][:, c0:],
                                                 func=Exp, scale=0.125)
                            if diag:
                                nc.gpsimd.affine_select(
                                    out=ET[:, c0:c0 + 128],
                                    in_=ET[:, c0:c0 + 128],
                                    compare_op=mybir.AluOpType.is_ge, fill=0.0,
                                    base=0, channel_multiplier=-1,
                                    pattern=[[1, 128]])
                            ETs.append(ET)
                        # M=128 lhsT covers both heads; each head's valid half
                        # lands on its own partitions
                        for hl in (0, 1):
                            nc.tensor.matmul(cts[hl][:, c0:],
                                             lhsT=V[:, i, P * k:P * (k + 1)],
                                             rhs=ETs[hl][:, c0:],
                                             start=(i == 0), stop=(i == n_i - 1))
                    for hl in (0, 1):
                        po = 64 * hl
                        nc.vector.tensor_mul(
                            out=ctxT_pair[po:po + 64, 512 * jw:512 * (jw + 1)],
                            in0=cts[hl][po:po + 64, :],
                            in1=rbc[po:po + 64, 512 * jw:512 * (jw + 1)])
                nc.sync.dma_start(out=ctxT_dram[P * k:P * (k + 1), :],
                                  in_=ctxT_pair)

        # --------- Phase 3: output projection (qkv pools freed) -----------
        with ExitStack() as ph:
            px2 = ph.enter_context(tc.tile_pool(name="px2", bufs=3))
            pbig = ph.enter_context(tc.tile_pool(name="pbig", bufs=1))
            pout = ph.enter_context(tc.tile_pool(name="pout", bufs=4))
            pst2 = ph.enter_context(tc.tile_pool(name="pst2", bufs=4,
                                                 space="PSUM"))
            psP = ph.enter_context(tc.tile_pool(name="psP", bufs=2,
                                                space="PSUM"))

            WprojT = pbig.tile([P, NK, C], DT)  # [c_in, c_in chunk, c_out]
            ctxT = pbig.tile([P, NK, T], DT)    # [hd, hd chunk, t]
            for ck in range(NK):
                nc.sync.dma_start(out=ctxT[:, ck, :],
                                  in_=ctxT_dram[P * ck:P * (ck + 1), :])
            bias_bc = pbig.tile([P, C], f32)
            bp_ap = bp_d[:]
            nc.gpsimd.dma_start(
                out=bias_bc,
                in_=bass.AP(tensor=bp_ap.tensor, offset=bp_ap.offset,
                            ap=[[0, P]] + [list(p) for p in bp_ap.ap]))

            for ci in range(NK):
                chunk = px2.tile([P, C], f32, tag="chunk")
                nc.sync.dma_start(out=chunk, in_=wp_d[P * ci:P * (ci + 1), :])
                for ki in range(NK):
                    pt = pst2.tile([P, P], f32, tag="tps2")
                    nc.tensor.transpose(pt, chunk[:, P * ki:P * (ki + 1)],
                                        id_sb)
                    nc.vector.tensor_copy(out=WprojT[:, ki, P * ci:P * (ci + 1)],
                                          in_=pt)
            for ti in range(NT):
                for cw in range(NW):
                    pt = psP.tile([P, 512], f32, tag="prps")
                    for ki in range(NK):
                        nc.tensor.matmul(pt, lhsT=ctxT[:, ki, P * ti:P * (ti + 1)],
                                         rhs=WprojT[:, ki,
                                                    512 * cw:512 * (cw + 1)],
                                         start=(ki == 0), stop=(ki == NK - 1))
                    ot = pout.tile([P, 512], f32, tag="ot")
                    nc.vector.tensor_add(out=ot, in0=pt,
                                         in1=bias_bc[:, 512 * cw:512 * (cw + 1)])
                    nc.sync.dma_start(
                        out=out_d[P * ti:P * (ti + 1), 512 * cw:512 * (cw + 1)],
                        in_=ot)


def kernel(x, Wq, Wk, Wv, Wproj, bproj):
    x = np.ascontiguousarray(np.asarray(x, dtype=np.float32))
    Wq = np.ascontiguousarray(np.asarray(Wq, dtype=np.float32))
    Wk = np.ascontiguousarray(np.asarray(Wk, dtype=np.float32))
    Wv = np.ascontiguousarray(np.asarray(Wv, dtype=np.float32))
    Wproj = np.ascontiguousarray(np.asarray(Wproj, dtype=np.float32))
    bproj = np.ascontiguousarray(np.asarray(bproj, dtype=np.float32))

    if "nc" not in _nc_cache:
        _nc_cache["nc"] = build()
    nc = _nc_cache["nc"]

    ident = np.eye(P, dtype=np.float32)
    bandmask = np.where(np.arange(P)[None, :] <= np.arange(P)[:, None],
                        0.0, -1e30).astype(np.float32)
    in_maps = [
        {"x": x[b], "Wq": Wq, "Wk": Wk, "Wv": Wv, "Wproj": Wproj,
         "bproj": bproj, "ident": ident, "bandmask": bandmask}
        for b in range(B)
    ]
    res = run_bass_kernel_spmd(nc, in_maps, list(range(B)))
    out = np.stack([res.results[b]["out"] for b in range(B)])
    wei = np.stack([res.results[b]["wei"] for b in range(B)])
    return (out, wei)


# revision 18
# speedup vs baseline: 46493.0961x; 11.8691x over previous
"""Multi-head causal attention (B=8, T=1024, C=1024, H=16, D=64) on 8 trn2 cores.

Sharding: data-parallel over batch B — core b computes batch element b fully
(QKV projections, causal softmax attention, output projection). No collectives.

Per-core program (all shapes hardcoded):
  xT = transpose(x)                       via PE-transpose (identity matmul)
  QT[hd, t], KT[hd, t] = W.T @ xT         (heads packed in pairs of 64 partitions)
  V[s, hd] = x @ Wv                       (natural layout, xT as lhsT)
  per head:
    S[t, s]  = QT_h.T @ KT_h  tiles       -> exp(S/8) (+row sums via accum_out)
                                          -> causal mask -> wei = E/rowsum -> DMA
    ST[s, t] = KT_h.T @ QT_h  tiles       -> exp -> mask   (E^T, unnormalized)
    ctx[t, d] = sum_s E^T[s, t-slice].T @ V[s, d]; normalized by 1/rowsum at the
                PSUM->SBUF copy (per-partition activation scale)
  ctx bounced to DRAM scratch; phase 3 reloads, PE-transposes, and computes
  out = ctx @ Wproj.T + bproj.

Above-diagonal tiles of `wei` are never written: output buffers are pre-zeroed
(donated zero buffers in the PJRT path), which the kernel relies on.

Precision: KERNEL_F32R=1 (default) runs all matmuls in float32r (full-rate PE,
~1 cycle/row; measured end-to-end error 2.6e-4 of absmax). KERNEL_F32R=0 runs
pure fp32 (4 cycles/row, ~2.3x slower; error 2.7e-6 of absmax).
"""

import os
import sys

sys.path.insert(0, "/opt/trn_rl_repo")

from contextlib import ExitStack

import numpy as np

import concourse.bass as bass
import concourse.tile as tile
from concourse import bacc, mybir
from concourse.bass_utils import run_bass_kernel_spmd

f32 = mybir.dt.float32
f32r = mybir.dt.float32r

B, T, C, H, D = 8, 1024, 1024, 16, 64
P = 128
NT = T // P        # 8 row chunks of 128
NK = C // P        # 8 contraction chunks of 128
NW = T // 512      # 2 free-dim windows of 512
HP = H // 2        # 8 head pairs

USE_F32R = os.environ.get("KERNEL_F32R", "1") == "1"
DT = f32r if USE_F32R else f32

Exp = mybir.ActivationFunctionType.Exp
Copy = mybir.ActivationFunctionType.Copy
X_AXIS = mybir.AxisListType.X

_nc_cache = {}


def _w_dma(nc):
    return nc.sync


def build(rep: int = 1, bench_io: bool = False):
    nc = bacc.Bacc("TRN2", target_bir_lowering=False, debug=False,
                   enable_asserts=True, num_devices=8)
    x_d = nc.declare_dram_parameter("x", [T, C], f32, isOutput=False)
    wq_d = nc.declare_dram_parameter("Wq", [H, C, D], DT, isOutput=False)
    wk_d = nc.declare_dram_parameter("Wk", [H, C, D], DT, isOutput=False)
    wv_d = nc.declare_dram_parameter("Wv", [H, C, D], DT, isOutput=False)
    wp_d = nc.declare_dram_parameter("Wproj", [C, C], f32, isOutput=False)
    bp_d = nc.declare_dram_parameter("bproj", [C], f32, isOutput=False)
    id_d = nc.declare_dram_parameter("ident", [P, P], f32, isOutput=False)
    bm_d = nc.declare_dram_parameter("bandmask", [P, P], f32, isOutput=False)
    if bench_io:
        # timing-only build: big outputs land in internal DRAM (same DMA
        # cost), external output is a tiny token
        out_d = nc.dram_tensor("out", [T, C], f32)
        wei_d = nc.dram_tensor("wei", [H, T, T], f32)
        tok_d = nc.declare_dram_parameter("tok", [1, 16], f32, isOutput=True)
    else:
        out_d = nc.declare_dram_parameter("out", [T, C], f32, isOutput=True)
        wei_d = nc.declare_dram_parameter("wei", [H, T, T], f32, isOutput=True)

    ctxT_dram = nc.dram_tensor("ctxT_scratch", [C, T], DT)
    rows_dram = nc.dram_tensor("recip_rows", [H, T], f32)

    with tile.TileContext(nc) as tc, ExitStack() as top:
        glob = top.enter_context(tc.tile_pool(name="glob", bufs=1))
        id_sb = glob.tile([P, P], f32)
        nc.sync.dma_start(out=id_sb, in_=id_d[:])
        bm_sb = glob.tile([P, P], f32)
        nc.sync.dma_start(out=bm_sb, in_=bm_d[:])

        for _ in range(rep):
            _body(nc, tc, x_d, wq_d, wk_d, wv_d, wp_d, bp_d, id_sb, bm_sb,
                  out_d, wei_d, ctxT_dram, rows_dram)
        if bench_io:
            nc.sync.dma_start(out=tok_d[:], in_=id_sb[0:1, 0:16])

    nc.compile()
    return nc


def _body(nc, tc, x_d, wq_d, wk_d, wv_d, wp_d, bp_d, id_sb, bm_sb, out_d,
          wei_d, ctxT_dram, rows_dram):
    with ExitStack() as live:
        qkv = live.enter_context(tc.tile_pool(name="qkv", bufs=1))
        V = qkv.tile([P, NT, H * D], DT)  # [s within chunk, s chunk, (h d)]
        xT = qkv.tile([P, NK, T], DT)     # [c within chunk, c chunk, t]

        # ------- Phase 1a: x transpose + V projection ----------------------
        with ExitStack() as ph:
            px = ph.enter_context(tc.tile_pool(name="px", bufs=2))
            pst = ph.enter_context(tc.tile_pool(name="pst", bufs=2, space="PSUM"))

            for ti in range(NT):
                xt = px.tile([P, C], f32, tag="xchunk")
                nc.sync.dma_start(out=xt, in_=x_d[P * ti:P * (ti + 1), :])
                for ci in range(NK):
                    pt = pst.tile([P, P], f32, tag="tps")
                    nc.tensor.transpose(pt, xt[:, P * ci:P * (ci + 1)], id_sb)
                    nc.vector.tensor_copy(out=xT[:, ci, P * ti:P * (ti + 1)],
                                          in_=pt)

        # ------- Phase 2: per pair: Q/K projection then attention ----------
        with ExitStack() as ph:
            pqk = ph.enter_context(tc.tile_pool(name="pqk", bufs=2))
            pwv = ph.enter_context(tc.tile_pool(name="pwv", bufs=9))
            pwt = ph.enter_context(tc.tile_pool(name="pwt", bufs=16))
            pcp = ph.enter_context(tc.tile_pool(name="pcp", bufs=3))
            pE = ph.enter_context(tc.tile_pool(name="pE", bufs=10))
            pET = ph.enter_context(tc.tile_pool(name="pET", bufs=8))
            pW = ph.enter_context(tc.tile_pool(name="pWout", bufs=4))
            prs = ph.enter_context(tc.tile_pool(name="prs", bufs=10))
            prc = ph.enter_context(tc.tile_pool(name="prc", bufs=4))
            prow = ph.enter_context(tc.tile_pool(name="prow", bufs=2))
            pbc = ph.enter_context(tc.tile_pool(name="pbc", bufs=3))
            psQK = ph.enter_context(tc.tile_pool(name="psQK", bufs=2,
                                                 space="PSUM"))
            psM = ph.enter_context(tc.tile_pool(name="psM", bufs=4, space="PSUM"))
            psCT = ph.enter_context(tc.tile_pool(name="psCT", bufs=2,
                                                 space="PSUM"))

            def project_qk(k):
                QTp = pqk.tile([P, T], DT, tag="qt", name=f"qt{k}")
                KTp = pqk.tile([P, T], DT, tag="kt", name=f"kt{k}")
                for w_d, OUT in ((wq_d, QTp), (wk_d, KTp)):
                    wts = []
                    for ci in range(NK):
                        wt = pwt.tile([P, 2, D], DT, tag="wqk")
                        nc.sync.dma_start(
                            out=wt,
                            in_=w_d[2 * k:2 * k + 2, P * ci:P * (ci + 1), :]
                            .rearrange("h c d -> c h d"))
                        wts.append(wt)
                    for tw in range(NW):
                        pt = psQK.tile([P, 512], f32, tag="qkps")
                        for ci in range(NK):
                            nc.tensor.matmul(pt, lhsT=wts[ci],
                                             rhs=xT[:, ci, 512 * tw:512 * (tw + 1)],
                                             start=(ci == 0), stop=(ci == NK - 1))
                        nc.vector.tensor_copy(
                            out=OUT[:, 512 * tw:512 * (tw + 1)], in_=pt)
                return QTp, KTp

            def compute_v(hw):
                wvts = []
                for ci in range(NK):
                    wvt = pwv.tile([P, 8, D], DT, tag="wvt")
                    nc.sync.dma_start(
                        out=wvt,
                        in_=wv_d[8 * hw:8 * hw + 8, P * ci:P * (ci + 1), :]
                        .rearrange("h c d -> c h d"))
                    wvts.append(wvt)
                for si in range(NT):
                    pt = psQK.tile([P, 512], f32, tag="qkps")
                    for ci in range(NK):
                        nc.tensor.matmul(pt, lhsT=xT[:, ci, P * si:P * (si + 1)],
                                         rhs=wvts[ci], start=(ci == 0),
                                         stop=(ci == NK - 1))
                    nc.vector.tensor_copy(out=V[:, si, 512 * hw:512 * (hw + 1)],
                                          in_=pt)

            pend = project_qk(0)
            compute_v(0)
            for k in range(HP):
                QTp, KTp = pend
                if k + 1 < HP:
                    pend = project_qk(k + 1)
                if k == 2:
                    # hd-window 1 of V is first needed by pair 4
                    compute_v(1)

                # normalized ctxT rows for this head pair, [hd within pair, t]
                ctxT_pair = pcp.tile([P, T], DT, tag="cpair")
                # per-head 1/rowsum broadcast rows: 0-63 even, 64-127 odd
                rbc = pbc.tile([P, T], f32, tag="rbc")
                QThs = [QTp[64 * hl:64 * hl + 64, :] for hl in (0, 1)]
                KThs = [KTp[64 * hl:64 * hl + 64, :] for hl in (0, 1)]
                recips = [prc.tile([P, NT], f32, tag="recip",
                                   name=f"recip{k}_{hl}") for hl in (0, 1)]

                # --- S side, both heads interleaved (concurrent row groups)
                for i in range(NT):
                    jd = i // 4           # diagonal 512-window index
                    m = i % 4             # 128-band position in the window
                    band_end = 128 * m + 128
                    rs_fin = [None, None]
                    E_all = [[], []]
                    for j in range(jd + 1):
                        pts = []
                        for hl in (0, 1):
                            pt = psM.tile([P, 512], f32, tag="mps")
                            nc.tensor.matmul(
                                pt, lhsT=QThs[hl][:, P * i:P * (i + 1)],
                                rhs=KThs[hl][:, 512 * j:512 * (j + 1)],
                                start=True, stop=True)
                            pts.append(pt)
                        for hl in (0, 1):
                            pt = pts[hl]
                            E = pE.tile([P, 512], f32, tag="E")
                            rs = prs.tile([P, 1], f32, tag="rs")
                            if j < jd:
                                nc.scalar.activation(out=E, in_=pt, func=Exp,
                                                     scale=0.125, accum_out=rs)
                            else:
                                nc.vector.tensor_add(
                                    out=pt[:, 128 * m:band_end],
                                    in0=pt[:, 128 * m:band_end], in1=bm_sb)
                                nc.scalar.activation(out=E[:, :band_end],
                                                     in_=pt[:, :band_end],
                                                     func=Exp, scale=0.125,
                                                     accum_out=rs)
                                if rs_fin[hl] is not None:
                                    nc.vector.tensor_add(out=rs, in0=rs,
                                                         in1=rs_fin[hl])
                            rs_fin[hl] = rs
                            E_all[hl].append(E)
                    for hl in (0, 1):
                        nc.vector.reciprocal(out=recips[hl][:, i:i + 1],
                                             in_=rs_fin[hl])
                        Wt = pW.tile([P, T], f32, tag="Wt")
                        for j, E in enumerate(E_all[hl]):
                            ncols = 512 if j < jd else band_end
                            nc.vector.tensor_scalar_mul(
                                Wt[:, 512 * j:512 * j + ncols], E[:, :ncols],
                                recips[hl][:, i:i + 1])
                        width = 512 * jd + band_end
                        nc.sync.dma_start(
                            out=wei_d[2 * k + hl, P * i:P * (i + 1), :width],
                            in_=Wt[:, :width])

                # 1/rowsum -> [1, T] rows in DRAM -> partition-broadcast
                for hl in (0, 1):
                    rp_ps = psM.tile([P, 512], f32, tag="mps",
                                     name=f"rps{k}_{hl}")
                    nc.tensor.transpose(rp_ps[:NT, :P], recips[hl], id_sb)
                    rp_sb = prow.tile([NT, P], f32, tag="rpsb")
                    nc.vector.tensor_copy(out=rp_sb, in_=rp_ps[:NT, :P])
                    nc.gpsimd.dma_start(out=rows_dram[2 * k + hl, :], in_=rp_sb)
                    row_ap = rows_dram[2 * k + hl, :]
                    nc.gpsimd.dma_start(
                        out=rbc[64 * hl:64 * hl + 64, :],
                        in_=bass.AP(tensor=row_ap.tensor, offset=row_ap.offset,
                                    ap=[[0, 64]] + [list(p) for p in row_ap.ap]))

                # --- ST side + PV, heads interleaved
                for jw in range(NW):
                    n_i = 4 * (jw + 1)
                    cts = [psCT.tile([P, 512], f32, tag="ctps",
                                     name=f"ct{k}_{jw}_{hl}") for hl in (0, 1)]
                    for i in range(n_i):
                        pts = []
                        for hl in (0, 1):
                            pt = psM.tile([P, 512], f32, tag="mps")
                            nc.tensor.matmul(
                                pt, lhsT=KThs[hl][:, P * i:P * (i + 1)],
                                rhs=QThs[hl][:, 512 * jw:512 * (jw + 1)],
                                start=True, stop=True)
                            pts.append(pt)
                        diag = i // 4 == jw
                        c0 = 128 * (i % 4) if diag else 0
                        ETs = []
                        for hl in (0, 1):
                            ET = pET.tile([P, 512], DT, tag="ET")
                            nc.scalar.activation(out=ET[:, c0:],
                                                 in_=pts[hl][:, c0:],
                                                 func=Exp, scale=0.125)
                            if diag:
                                nc.gpsimd.affine_select(
                                    out=ET[:, c0:c0 + 128],
                                    in_=ET[:, c0:c0 + 128],
                                    compare_op=mybir.AluOpType.is_ge, fill=0.0,
                                    base=0, channel_multiplier=-1,
                                    pattern=[[1, 128]])
                            ETs.append(ET)
                        # M=128 lhsT covers both heads; each head's valid half
                        # lands on its own partitions
                        for hl in (0, 1):
                            nc.tensor.matmul(cts[hl][:, c0:],
                                             lhsT=V[:, i, P * k:P * (k + 1)],
                                             rhs=ETs[hl][:, c0:],
                                             start=(i == 0), stop=(i == n_i - 1))
                    for hl in (0, 1):
                        po = 64 * hl
                        nc.vector.tensor_mul(
                            out=ctxT_pair[po:po + 64, 512 * jw:512 * (jw + 1)],
                            in0=cts[hl][po:po + 64, :],
                            in1=rbc[po:po + 64, 512 * jw:512 * (jw + 1)])
                nc.sync.dma_start(out=ctxT_dram[P * k:P * (k + 1), :],
                                  in_=ctxT_pair)

        # --------- Phase 3: output projection (qkv pools freed) -----------
        with ExitStack() as ph:
            px2 = ph.enter_context(tc.tile_pool(name="px2", bufs=3))
            pbig = ph.enter_context(tc.tile_pool(name="pbig", bufs=1))
            pout = ph.enter_context(tc.tile_pool(name="pout", bufs=4))
            pst2 = ph.enter_context(tc.tile_pool(name="pst2", bufs=4,
                                                 space="PSUM"))
            psP = ph.enter_context(tc.tile_pool(name="psP", bufs=2,
                                                space="PSUM"))

            WprojT = pbig.tile([P, NK, C], DT)  # [c_in, c_in chunk, c_out]
            ctxT = pbig.tile([P, NK, T], DT)    # [hd, hd chunk, t]
            for ck in range(NK):
                nc.sync.dma_start(out=ctxT[:, ck, :],
                                  in_=ctxT_dram[P * ck:P * (ck + 1), :])
            bias_bc = pbig.tile([P, C], f32)
            bp_ap = bp_d[:]
            nc.gpsimd.dma_start(
                out=bias_bc,
                in_=bass.AP(tensor=bp_ap.tensor, offset=bp_ap.offset,
                            ap=[[0, P]] + [list(p) for p in bp_ap.ap]))

            for ci in range(NK):
                chunk = px2.tile([P, C], f32, tag="chunk")
                nc.sync.dma_start(out=chunk, in_=wp_d[P * ci:P * (ci + 1), :])
                for ki in range(NK):
                    pt = pst2.tile([P, P], f32, tag="tps2")
                    nc.tensor.transpose(pt, chunk[:, P * ki:P * (ki + 1)],
                                        id_sb)
                    nc.vector.tensor_copy(out=WprojT[:, ki, P * ci:P * (ci + 1)],
                                          in_=pt)
            for ti in range(NT):
                for cw in range(NW):
                    pt = psP.tile([P, 512], f32, tag="prps")
                    for ki in range(NK):
                        nc.tensor.matmul(pt, lhsT=ctxT[:, ki, P * ti:P * (ti + 1)],
                                         rhs=WprojT[:, ki,
                                                    512 * cw:512 * (cw + 1)],
                                         start=(ki == 0), stop=(ki == NK - 1))
                    ot = pout.tile([P, 512], f32, tag="ot")
                    nc.vector.tensor_add(out=ot, in0=pt,
                                         in1=bias_bc[:, 512 * cw:512 * (cw + 1)])
                    nc.sync.dma_start(
                        out=out_d[P * ti:P * (ti + 1), 512 * cw:512 * (cw + 1)],
                        in_=ot)


def kernel(x, Wq, Wk, Wv, Wproj, bproj):
    x = np.ascontiguousarray(np.asarray(x, dtype=np.float32))
    Wq = np.ascontiguousarray(np.asarray(Wq, dtype=np.float32))
    Wk = np.ascontiguousarray(np.asarray(Wk, dtype=np.float32))
    Wv = np.ascontiguousarray(np.asarray(Wv, dtype=np.float32))
    Wproj = np.ascontiguousarray(np.asarray(Wproj, dtype=np.float32))
    bproj = np.ascontiguousarray(np.asarray(bproj, dtype=np.float32))

    if "nc" not in _nc_cache:
        _nc_cache["nc"] = build()
    nc = _nc_cache["nc"]

    ident = np.eye(P, dtype=np.float32)
    bandmask = np.where(np.arange(P)[None, :] <= np.arange(P)[:, None],
                        0.0, -1e30).astype(np.float32)
    in_maps = [
        {"x": x[b], "Wq": Wq, "Wk": Wk, "Wv": Wv, "Wproj": Wproj,
         "bproj": bproj, "ident": ident, "bandmask": bandmask}
        for b in range(B)
    ]
    res = run_bass_kernel_spmd(nc, in_maps, list(range(B)))
    out = np.stack([res.results[b]["out"] for b in range(B)])
    wei = np.stack([res.results[b]["wei"] for b in range(B)])
    return (out, wei)
